# revision 30
# baseline (speedup 1.0000x reference)
"""BitSwiGLU Trainium2 kernel (8 NeuronCores, data-parallel tokens +
distributed weight ternarization with AllGather of ternary weights).

Math (per bit_linear, forward values):
    gamma_x = clip(max|x_row|, 1e-5);  k = rne(x * 127/gamma_x)  in [-127,127]
    gamma_w = clip(mean|w|, 1e-5);    t = sign(w) * (|w| > 0.5*gamma_w)  in {-1,0,1}
    y = (k @ t.T) * (gamma_x*gamma_w/127) + b

k and t are small integers, exactly representable in bf16; the TensorEngine
accumulates bf16 products in fp32 PSUM, so k @ t.T is EXACT integer math at
bf16 speed. All scales are applied per-token (per-partition) at PSUM eviction.
Ternarization runs as t2 = sign(w-thr) + sign(w+thr) in {-2,0,2}; the factor
2 is folded into the eviction scales.

Sharding: data-parallel over tokens (8192 -> 1024/core) for the matmuls.
Weight ternarization is DISTRIBUTED: core i ternarizes gate/val rows
[i*1024:(i+1)*1024] and out_w columns [i*1024:(i+1)*1024] (host passes only
the shard), then AllGathers replicate the bf16 ternary weights. The ternary
weights are stored PRE-TRANSPOSED (contraction dim on partitions) via
SBUF-side DMA transposes during ternarize, so the matmul phases use large
natural DMA loads. gamma = mean|w| is per-core partials + one tiny
AllReduce ([128,3]); a dummy warmup AllReduce absorbs the collective-stack
cold start.

h (the mm1 output) is staged in DRAM as fp16 (it is re-quantized to int8
levels for mm2 anyway, so fp16 rounding is far below the rel-err gate).

Gathered ternary layouts (rank-major hidden order == natural global order):
  gv{j}_gat[r, w, kd, p, h512]: w=0 gate / w=1 val, d = kd*128 + p,
      global hidden row = r*1024 + j*512 + h512
  oq_gat[r, kh, p, d]: global hidden (mm2 contraction) = r*1024 + kh*128 + p
"""

import numpy as np

import concourse.bass as bass
import concourse.mybir as mybir
import concourse.tile as tile
from concourse import bacc
from concourse import bass_isa
from concourse.bass_utils import run_bass_kernel_spmd

F32 = mybir.dt.float32
F16 = mybir.dt.float16
BF16 = mybir.dt.bfloat16
AF = mybir.ActivationFunctionType
OP = mybir.AluOpType
AX = mybir.AxisListType

MAGIC = 12582912.0  # 1.5 * 2**23 : (v + MAGIC) - MAGIC == rne(v) for |v| < 2**22

N_CORES = 8
RGRP = [[0, 1, 2, 3, 4, 5, 6, 7]]


def _build(T, D, H, n_cores=N_CORES, fp8=True):
    nc = bacc.Bacc("TRN2", target_bir_lowering=False, debug=False,
                   num_devices=n_cores)
    HS = H // n_cores            # gate/val row shard per core
    # host passes the weight shards PRE-TRANSPOSED (contraction-major):
    #   gate_wT/val_wT: [D, HS], out_wT: [HS, D]
    x_d = nc.dram_tensor("x", [T, D], F32, kind="ExternalInput")
    gw_d = nc.dram_tensor("gate_wT", [D, HS], F32, kind="ExternalInput")
    vw_d = nc.dram_tensor("val_wT", [D, HS], F32, kind="ExternalInput")
    ow_d = nc.dram_tensor("out_wT", [HS, D], F32, kind="ExternalInput")
    out_d = nc.dram_tensor("out", [T, D], F32, kind="ExternalOutput")

    with tile.TileContext(nc) as tc:
        _body(tc, x_d, gw_d, vw_d, ow_d, out_d, T=T, D=D, H=H,
              n_cores=n_cores, fp8=fp8)
    nc.compile()
    return nc


def _body(tc, x_d, gw_d, vw_d, ow_d, out_d, *, T, D, H, n_cores, fp8):
    nc = tc.nc
    KD = D // 128      # contraction chunks, mm1 (16)
    KH = H // 128      # contraction chunks, mm2 (64)
    NH = H // 512      # hidden 512-chunks (mm1 output tiles) (16)
    ND = D // 512      # d_out 512-chunks (mm2 output tiles) (4)
    MT = T // 128      # token chunks (8)
    HS = H // n_cores  # own gate/val rows (1024)
    RG = HS // 128     # own gate/val row-chunks (8)
    RO = D // 128      # own out_w d-row chunks (16)
    CQ = 2048          # h-quant processing chunk
    NQ = H // CQ
    MHALF = max(1, MT // 2)

    Xv = x_d.ap().rearrange("(m p) d -> m p d", p=128)
    Ov = out_d.ap().rearrange("(m p) d -> m p d", p=128)

    with (
        tc.tile_pool(name="persist", bufs=1) as pp,
        tc.tile_pool(name="psp", bufs=8, space="PSUM") as psp,
        tc.tile_pool(name="drp", bufs=1, space="DRAM") as drp,
    ):
        # DRAM scratch.  own ternary (AG inputs, pre-transposed):
        #   gv{j}_own[w, kd, p, h512]   (w: 0=gate, 1=val)
        #   oq_own[kh_local, p, d]      (own hidden cols of out_w)
        TDT = mybir.dt.float8e4 if fp8 else BF16   # gv ternary transport
        gv0_own = drp.tile([2, KD, 128, 512], TDT, tag="gv0_own")
        gv1_own = drp.tile([2, KD, 128, 512], TDT, tag="gv1_own")
        oq_own = drp.tile([HS // 128, 128, D], TDT, tag="oq_own")
        gv0_gat = drp.tile([n_cores, 2, KD, 128, 512], TDT, tag="gv0_gat",
                           addr_space="Shared")
        gv1_gat = drp.tile([n_cores, 2, KD, 128, 512], TDT, tag="gv1_gat",
                           addr_space="Shared")
        oq_gat = drp.tile([n_cores, HS // 128, 128, D], TDT, tag="oq_gat",
                          addr_space="Shared")
        ar_in = drp.tile([128, 3], F32, tag="ar_in")
        ar_out = drp.tile([128, 3], F32, tag="ar_out", addr_space="Shared")
        warm_in = drp.tile([1, 4], F32, tag="warm_in")
        warm_out = drp.tile([1, 4], F32, tag="warm_out", addr_space="Shared")
        h_d = drp.tile([MT, 128, H], F16, tag="h")

        s1, s12, gx_l, hmax = [], [], [], []
        for m in range(MT):
            for nm, lst in (("s1", s1), ("s12", s12), ("gx", gx_l),
                            ("hmax", hmax)):
                t = pp.tile([128, 1], F32, tag=f"{nm}{m}", name=f"{nm}{m}")
                lst.append(t)
        hp = [pp.tile([128, NH], F32, tag=f"hp{m}", name=f"hp{m}")
              for m in range(MT)]
        parts = pp.tile([128, 40], F32, tag="parts")
        sums = pp.tile([128, 3], F32, tag="sums")
        gsb = pp.tile([128, 3], F32, tag="gsb")
        g3 = pp.tile([128, 3], F32, tag="g3")
        thr3 = pp.tile([128, 3], F32, tag="thr3")
        nthr3 = pp.tile([128, 3], F32, tag="nthr3")

        Gv = gw_d.ap().rearrange("(r p) c -> r p c", p=128)  # [KD,128,HS]
        Vv = vw_d.ap().rearrange("(r p) c -> r p c", p=128)  # [KD,128,HS]
        Wo = ow_d.ap().rearrange("(r p) c -> r p c", p=128)  # [HS/128,128,D]

        # read views for the matmul phases
        gv0_rd = gv0_gat[:, :, :, :, :].rearrange("r w k p h -> r w p k h")
        gv1_rd = gv1_gat[:, :, :, :, :].rearrange("r w k p h -> r w p k h")
        oq_rd = oq_gat[:, :, :, :].rearrange("r k p d -> r p k d")

        with tc.tile_pool(name="kxp", bufs=1) as kxp:
            kxT = kxp.tile([128, KD, T], BF16, tag="kxT")

            # warmup collective: absorbs the CC-stack cold start while the
            # gamma DMAs stream
            with tc.tile_pool(name="wrm", bufs=1) as wrm:
                wz = wrm.tile([1, 4], F32, tag="wz")
                nc.vector.memset(wz[:, :], 0.0)
                nc.sync.dma_start(out=warm_in[:, :], in_=wz[:, :])
                nc.gpsimd.collective_compute(
                    "AllReduce", OP.add, ins=[warm_in[:, :]],
                    outs=[warm_out[:, :]], replica_groups=RGRP)

            # ------------- gamma partials + AllReduce ---------------------
            with tc.tile_pool(name="gp", bufs=3) as gp:
                def abs_chunk(src, col, W, tg):
                    wt = gp.tile([128, W], F32, tag=f"{tg}_in")
                    nc.sync.dma_start(out=wt[:, :], in_=src)
                    scr = gp.tile([128, W], BF16, tag=f"{tg}_scr", bufs=2)
                    nc.scalar.activation(out=scr[:, :], in_=wt[:, :],
                                         func=AF.Abs,
                                         accum_out=parts[:, col:col + 1])

                for r in range(KD):
                    abs_chunk(Gv[r], r, HS, "g")
                for r in range(KD):
                    abs_chunk(Vv[r], 16 + r, HS, "g")
                for r in range(HS // 128):
                    abs_chunk(Wo[r], 32 + r, D, "o")
                nc.vector.tensor_reduce(out=sums[:, 0:1], in_=parts[:, 0:16],
                                        axis=AX.X, op=OP.add)
                nc.vector.tensor_reduce(out=sums[:, 1:2],
                                        in_=parts[:, 16:32],
                                        axis=AX.X, op=OP.add)
                nc.vector.tensor_reduce(out=sums[:, 2:3],
                                        in_=parts[:, 32:40],
                                        axis=AX.X, op=OP.add)
                nc.sync.dma_start(out=ar_in[:, :], in_=sums[:, :])

            # ------------- x quantization + transpose ---------------------
            # kxT[p=d, k, t] = k_x[t, k*128+p]
            with tc.tile_pool(name="xp", bufs=3) as xp:
                for m in range(MT):
                    xt = xp.tile([128, D], F32, tag="x_in")
                    nc.sync.dma_start(out=xt[:, :], in_=Xv[m])
                    gx = gx_l[m]
                    nc.vector.tensor_reduce(out=gx[:, :], in_=xt[:, :],
                                            axis=AX.X, op=OP.max,
                                            apply_absolute_value=True)
                    nc.vector.tensor_scalar_max(out=gx[:, :], in0=gx[:, :],
                                                scalar1=1e-5)
                    rcp = xp.tile([128, 1], F32, tag="rcpx")
                    nc.vector.reciprocal(out=rcp[:, :], in_=gx[:, :])
                    sx = xp.tile([128, 1], F32, tag="sx")
                    nc.vector.tensor_scalar_mul(out=sx[:, :], in0=rcp[:, :],
                                                scalar1=127.0)
                    xs = xp.tile([128, D], F32, tag="x_sc")
                    nc.scalar.activation(out=xs[:, :], in_=xt[:, :],
                                         func=AF.Copy, scale=sx[:, :])
                    kx = xp.tile([128, D], BF16, tag="kx")
                    nc.vector.tensor_scalar(out=kx[:, :], in0=xs[:, :],
                                            scalar1=MAGIC, scalar2=MAGIC,
                                            op0=OP.add, op1=OP.subtract)
                    nc.sync.dma_start(out=kxT[:, :, m * 128:(m + 1) * 128],
                                      in_=kx[:, :], transpose=True)

            # ------------- scales + ternarize own shards + AllGather ------
            with tc.tile_pool(name="wp", bufs=3) as wp:
                # gamma AllReduce (collective-stack warm by now)
                nc.gpsimd.collective_compute(
                    "AllReduce", OP.add, ins=[ar_in[:, :]],
                    outs=[ar_out[:, :]], replica_groups=RGRP)
                nc.sync.dma_start(out=gsb[:, :], in_=ar_out[:, :])
                nc.gpsimd.partition_all_reduce(gsb[:, :], gsb[:, :], 128,
                                               bass_isa.ReduceOp.add)
                # gamma = clip(mean, 1e-5); thr = 0.5*gamma
                nc.vector.tensor_scalar(out=g3[:, :], in0=gsb[:, :],
                                        scalar1=1.0 / (H * D),
                                        scalar2=1e-5, op0=OP.mult,
                                        op1=OP.max)
                nc.vector.tensor_scalar_mul(out=thr3[:, :], in0=g3[:, :],
                                            scalar1=0.5)
                nc.vector.tensor_scalar_mul(out=nthr3[:, :], in0=thr3[:, :],
                                            scalar1=-1.0)
                # per-token eviction scales; thr/127 == gamma/254 folds the
                # ternary 2x.  s12[m] here holds the VAL scale s2.
                for m in range(MT):
                    nc.vector.tensor_scalar(out=s1[m][:, :],
                                            in0=gx_l[m][:, :],
                                            scalar1=thr3[:, 0:1],
                                            scalar2=1.0 / 127.0,
                                            op0=OP.mult, op1=OP.mult)
                    nc.vector.tensor_scalar(out=s12[m][:, :],
                                            in0=gx_l[m][:, :],
                                            scalar1=thr3[:, 1:2],
                                            scalar2=1.0 / 127.0,
                                            op0=OP.mult, op1=OP.mult)

                # ternarize one [128, W] chunk -> {-2,0,2} tile (dt=odt);
                # loads ride the SWDGE/gpsimd queue to overlap the sync queue
                def tern_chunk(src, W, tg, thr, nthr, dve, odt=BF16,
                               out_ap=None, in_bufs=3):
                    wt = wp.tile([128, W], F32, tag=f"{tg}_in", bufs=in_bufs)
                    nc.sync.dma_start(out=wt[:, :], in_=src)
                    if out_ap is None:
                        tq = wp.tile([128, W], odt, tag=f"{tg}_tq")
                    else:
                        tq = None
                    if dve:
                        mp = wp.tile([128, W], BF16, tag=f"{tg}_mp", bufs=3)
                        nc.vector.tensor_scalar(out=mp[:, :], in0=wt[:, :],
                                                scalar1=thr, scalar2=2.0,
                                                op0=OP.is_gt, op1=OP.mult)
                        mn = wp.tile([128, W], BF16, tag=f"{tg}_mn", bufs=3)
                        nc.vector.tensor_scalar(out=mn[:, :], in0=wt[:, :],
                                                scalar1=nthr, scalar2=2.0,
                                                op0=OP.is_lt, op1=OP.mult)
                        dst = tq[:, :] if out_ap is None else out_ap
                        nc.vector.tensor_sub(out=dst, in0=mp[:, :],
                                             in1=mn[:, :])
                    else:
                        sp = wp.tile([128, W], BF16, tag=f"{tg}_sp", bufs=3)
                        nc.scalar.activation(out=sp[:, :], in_=wt[:, :],
                                             func=AF.Sign, bias=nthr)
                        sn = wp.tile([128, W], BF16, tag=f"{tg}_sn", bufs=3)
                        nc.scalar.activation(out=sn[:, :], in_=wt[:, :],
                                             func=AF.Sign, bias=thr)
                        dst = tq[:, :] if out_ap is None else out_ap
                        nc.vector.tensor_add(out=dst, in0=sp[:, :],
                                             in1=sn[:, :])
                    return tq

                t_g, nt_g = thr3[:, 0:1], nthr3[:, 0:1]
                t_v, nt_v = thr3[:, 1:2], nthr3[:, 1:2]
                t_o, nt_o = thr3[:, 2:3], nthr3[:, 2:3]

                # gate/val: natural writes, even/odd h-halves -> AG#0/AG#1
                for r in range(KD):
                    tqg = tern_chunk(Gv[r], HS, "g", t_g, nt_g, False,
                                     odt=TDT, in_bufs=10)
                    nc.sync.dma_start(out=gv0_own[0, r, :, :],
                                      in_=tqg[:, 0:HS // 2])
                    nc.sync.dma_start(out=gv1_own[0, r, :, :],
                                      in_=tqg[:, HS // 2:HS])
                    tqv = tern_chunk(Vv[r], HS, "g", t_v, nt_v, True,
                                     odt=TDT, in_bufs=10)
                    nc.sync.dma_start(out=gv0_own[1, r, :, :],
                                      in_=tqv[:, 0:HS // 2])
                    nc.sync.dma_start(out=gv1_own[1, r, :, :],
                                      in_=tqv[:, HS // 2:HS])
                nc.gpsimd.collective_compute(
                    "AllGather", OP.bypass, ins=[gv0_own[:, :, :, :]],
                    outs=[gv0_gat[:, :, :, :, :]], replica_groups=RGRP)
                nc.gpsimd.collective_compute(
                    "AllGather", OP.bypass, ins=[gv1_own[:, :, :, :]],
                    outs=[gv1_gat[:, :, :, :, :]], replica_groups=RGRP)

                # out_w own columns (pre-transposed): natural writes
                for r in range(HS // 128):
                    tqo = tern_chunk(Wo[r], D, "o", t_o, nt_o,
                                     dve=(r % 2 == 1), odt=TDT, in_bufs=4)
                    nc.sync.dma_start(out=oq_own[r, :, :], in_=tqo[:, :])
                nc.gpsimd.collective_compute(
                    "AllGather", OP.bypass, ins=[oq_own[:, :, :]],
                    outs=[oq_gat[:, :, :, :]], replica_groups=RGRP)

            # ---------------- mm1: gate/val matmuls + h ----------------
            gat_rd = [gv0_rd, gv1_rd]
            order = [2 * r for r in range(NH // 2)] + \
                    [2 * r + 1 for r in range(NH // 2)]
            with tc.tile_pool(name="m1p", bufs=2) as m1p:
                for n in order:
                    j, r = n % 2, n // 2
                    # natural bulk loads of pre-transposed weights; the
                    # fp8 ternary feeds the PE directly (mixed-dtype matmul)
                    wg_n = m1p.tile([128, KD, 512], TDT, tag="wg_n")
                    nc.sync.dma_start(out=wg_n[:, :, :],
                                      in_=gat_rd[j][r, 0])
                    wv_n = m1p.tile([128, KD, 512], TDT, tag="wv_n")
                    nc.sync.dma_start(out=wv_n[:, :, :],
                                      in_=gat_rd[j][r, 1])
                    for hf in range(MT // MHALF):
                        ms = range(hf * MHALF, (hf + 1) * MHALF)
                        pg = {m: psp.tile([128, 512], F32, tag="ps",
                                          name=f"pg{n}_{m}") for m in ms}
                        pv = {m: psp.tile([128, 512], F32, tag="ps",
                                          name=f"pv{n}_{m}") for m in ms}
                        for k in range(KD):
                            for m in ms:
                                lhsT = kxT[:, k, m * 128:(m + 1) * 128]
                                nc.tensor.matmul(pg[m][:, :], lhsT=lhsT,
                                                 rhs=wg_n[:, k, :],
                                                 start=(k == 0),
                                                 stop=(k == KD - 1))
                                nc.tensor.matmul(pv[m][:, :], lhsT=lhsT,
                                                 rhs=wv_n[:, k, :],
                                                 start=(k == 0),
                                                 stop=(k == KD - 1))
                        for m in ms:
                            # h = silu(pg*s1) * (pv*s2): one ACT + two DVE
                            A = m1p.tile([128, 512], F32, tag="Asb",
                                         bufs=MHALF + 2, name=f"A{n}_{m}")
                            nc.scalar.activation(out=A[:, :], in_=pg[m][:, :],
                                                 func=AF.Silu,
                                                 scale=s1[m][:, :])
                            tmp = m1p.tile([128, 512], F32, tag="tmp", bufs=4,
                                           name=f"tmp{n}_{m}")
                            nc.vector.tensor_scalar_mul(out=tmp[:, :],
                                                        in0=pv[m][:, :],
                                                        scalar1=s12[m][:, :])
                            hs = m1p.tile([128, 512], F16, tag="hsl", bufs=4,
                                          name=f"hs{n}_{m}")
                            nc.vector.tensor_mul(out=hs[:, :], in0=A[:, :],
                                                 in1=tmp[:, :])
                            nc.vector.tensor_reduce(
                                out=hp[m][:, n:n + 1], in_=hs[:, :],
                                axis=AX.X, op=OP.max,
                                apply_absolute_value=True)
                            nc.sync.dma_start(
                                out=h_d[m, :, n * 512:(n + 1) * 512],
                                in_=hs[:, :])

        # ---------------- h quantization + mm2 (interleaved) ----------
        with (
            tc.tile_pool(name="khp", bufs=1) as khp,
            tc.tile_pool(name="hqp", bufs=2) as hqp,
            tc.tile_pool(name="m2p", bufs=2) as m2p,
        ):
            khT, s_out = [], []
            NQT = 8                    # k-groups per c (one rank each)
            KQ = KH // NQT             # 8 k-chunks per group
            wo_pref = {}
            for q in range(2):         # prefetch c=0 weight tiles early
                wt0 = m2p.tile([128, KQ, 512], TDT, tag="wo_q", bufs=4,
                               name=f"wo_pref{q}")
                nc.sync.dma_start(out=wt0[:, :, :],
                                  in_=oq_rd[q][:, :, 0:512])
                wo_pref[(0, q)] = wt0
            for m in range(MT):
                nc.vector.tensor_reduce(out=hmax[m][:, :],
                                        in_=hp[m][:, :], axis=AX.X,
                                        op=OP.max)
                gh = hqp.tile([128, 1], F32, tag="gh")
                nc.vector.tensor_scalar_max(out=gh[:, :],
                                            in0=hmax[m][:, :],
                                            scalar1=1e-5)
                rch = hqp.tile([128, 1], F32, tag="rch")
                nc.vector.reciprocal(out=rch[:, :], in_=gh[:, :])
                sh = hqp.tile([128, 1], F32, tag="sh")
                nc.vector.tensor_scalar_mul(out=sh[:, :], in0=rch[:, :],
                                            scalar1=127.0)
                so = pp.tile([128, 1], F32, tag=f"so{m}", name=f"so{m}")
                nc.vector.tensor_scalar(out=so[:, :], in0=gh[:, :],
                                        scalar1=thr3[:, 2:3],
                                        scalar2=1.0 / 127.0,
                                        op0=OP.mult, op1=OP.mult)
                s_out.append(so)
                kT = khp.tile([128, KH, 128], BF16, tag=f"khT{m}",
                              name=f"khT{m}")
                khT.append(kT)
                # k_h = rne(h*sh); the (h*sh + MAGIC) pass alternates
                # between ScalarE and VectorE, and each kh half is
                # transposed as soon as it is complete
                kh_full = hqp.tile([128, H], BF16, tag="kh_full", bufs=1)
                for q in range(2):
                    HH = H // 2
                    hc = hqp.tile([128, HH], F16, tag="h_rd")
                    nc.scalar.dma_start(out=hc[:, :],
                                        in_=h_d[m, :, q * HH:(q + 1) * HH])
                    for u in range(HH // CQ):
                        sli = slice(u * CQ, (u + 1) * CQ)
                        slo = slice(q * HH + u * CQ, q * HH + (u + 1) * CQ)
                        tmp = hqp.tile([128, CQ], F32, tag="h_tmp")
                        if (q * (HH // CQ) + u) % 2 == 0:
                            nc.vector.tensor_scalar(out=tmp[:, :],
                                                    in0=hc[:, sli],
                                                    scalar1=sh[:, :],
                                                    scalar2=MAGIC,
                                                    op0=OP.mult, op1=OP.add)
                        else:
                            nc.scalar.activation(out=tmp[:, :],
                                                 in_=hc[:, sli],
                                                 func=AF.Copy,
                                                 scale=sh[:, :],
                                                 bias=MAGIC)
                        nc.vector.tensor_scalar_sub(out=kh_full[:, slo],
                                                    in0=tmp[:, :],
                                                    scalar1=MAGIC)
                    nc.sync.dma_start(
                        out=kT[:, q * (KH // 2):(q + 1) * (KH // 2), :],
                        in_=kh_full[:, q * HH:(q + 1) * HH],
                        transpose=True)

            # mm2: c outer, k-quarters, m-chains -- tensor engine starts as
            # soon as khT[0] is ready; wo loads are natural bulk reads
            for c in range(ND):
                csl = slice(c * 512, (c + 1) * 512)
                po = [psp.tile([128, 512], F32, tag="ps",
                               name=f"po{c}_{m}") for m in range(MT)]
                for q in range(NQT):
                    if (c, q) in wo_pref:
                        wo_q = wo_pref[(c, q)]
                    else:
                        wo_q = m2p.tile([128, KQ, 512], TDT, tag="wo_q",
                                        bufs=4)
                        nc.sync.dma_start(out=wo_q[:, :, :],
                                          in_=oq_rd[q][:, :, csl])
                    for m in range(MT):
                        for kk in range(KQ):
                            k = q * KQ + kk
                            nc.tensor.matmul(po[m][:, :],
                                             lhsT=khT[m][:, k, :],
                                             rhs=wo_q[:, kk, :],
                                             start=(k == 0),
                                             stop=(k == KH - 1))
                for m in range(MT):
                    ot = m2p.tile([128, 512], F32, tag="ot", bufs=4,
                                  name=f"ot{c}_{m}")
                    nc.scalar.activation(out=ot[:, :], in_=po[m][:, :],
                                         func=AF.Copy,
                                         scale=s_out[m][:, :])
                    nc.sync.dma_start(out=Ov[m][:, csl], in_=ot[:, :])


_NC_CACHE = {}


def _get_nc(T, D, H):
    key = (T, D, H)
    if key not in _NC_CACHE:
        try:
            _NC_CACHE[key] = _build(T, D, H, fp8=True)
        except Exception:
            _NC_CACHE[key] = _build(T, D, H, fp8=False)
    return _NC_CACHE[key]


def kernel(x, gate_w, gate_b, val_w, val_b, out_w, out_b, _trace=False):
    x = np.ascontiguousarray(np.asarray(x), dtype=np.float32)
    gate_w = np.ascontiguousarray(np.asarray(gate_w), dtype=np.float32)
    val_w = np.ascontiguousarray(np.asarray(val_w), dtype=np.float32)
    out_w = np.ascontiguousarray(np.asarray(out_w), dtype=np.float32)
    gate_b = np.asarray(gate_b)
    val_b = np.asarray(val_b)
    out_b = np.asarray(out_b)
    assert not np.any(gate_b) and not np.any(val_b), (
        "device kernel folds silu(y+b) with b=0; nonzero gate/val bias "
        "not supported")

    orig_shape = x.shape
    xf = x.reshape(-1, x.shape[-1])
    n_tok, d = xf.shape
    h = gate_w.shape[0]
    t_core = n_tok // N_CORES
    hs = h // N_CORES

    nc = _get_nc(t_core, d, h)
    in_maps = [
        {
            "x": xf[i * t_core:(i + 1) * t_core],
            "gate_wT": np.ascontiguousarray(gate_w[i * hs:(i + 1) * hs].T),
            "val_wT": np.ascontiguousarray(val_w[i * hs:(i + 1) * hs].T),
            "out_wT": np.ascontiguousarray(out_w[:, i * hs:(i + 1) * hs].T),
        }
        for i in range(N_CORES)
    ]
    res = run_bass_kernel_spmd(nc, in_maps, core_ids=list(range(N_CORES)),
                               trace=_trace)
    out = np.concatenate([res.results[i]["out"] for i in range(N_CORES)],
                         axis=0)
    out = out + out_b[None, :].astype(np.float32)
    kernel._last_results = res
    return out.reshape(orig_shape)


# revision 31
# speedup vs baseline: 1.1908x; 1.1908x over previous
"""BitSwiGLU Trainium2 kernel (8 NeuronCores, data-parallel tokens +
distributed weight ternarization with AllGather of ternary weights).

Math (per bit_linear, forward values):
    gamma_x = clip(max|x_row|, 1e-5);  k = rne(x * 127/gamma_x)  in [-127,127]
    gamma_w = clip(mean|w|, 1e-5);    t = sign(w) * (|w| > 0.5*gamma_w)  in {-1,0,1}
    y = (k @ t.T) * (gamma_x*gamma_w/127) + b

k and t are small integers, exactly representable in bf16; the TensorEngine
accumulates bf16 products in fp32 PSUM, so k @ t.T is EXACT integer math at
bf16 speed. All scales are applied per-token (per-partition) at PSUM eviction.
Ternarization runs as t2 = sign(w-thr) + sign(w+thr) in {-2,0,2}; the factor
2 is folded into the eviction scales.

Sharding: data-parallel over tokens (8192 -> 1024/core) for the matmuls.
Weight ternarization is DISTRIBUTED: core i ternarizes gate/val rows
[i*1024:(i+1)*1024] and out_w columns [i*1024:(i+1)*1024] (host passes only
the shard), then AllGathers replicate the bf16 ternary weights. The ternary
weights are stored PRE-TRANSPOSED (contraction dim on partitions) via
SBUF-side DMA transposes during ternarize, so the matmul phases use large
natural DMA loads. gamma = mean|w| is per-core partials + one tiny
AllReduce ([128,3]); a dummy warmup AllReduce absorbs the collective-stack
cold start.

h (the mm1 output) is staged in DRAM as fp16 (it is re-quantized to int8
levels for mm2 anyway, so fp16 rounding is far below the rel-err gate).

Gathered ternary layouts (rank-major hidden order == natural global order):
  gv{j}_gat[r, w, kd, p, h512]: w=0 gate / w=1 val, d = kd*128 + p,
      global hidden row = r*1024 + j*512 + h512
  oq_gat[r, kh, p, d]: global hidden (mm2 contraction) = r*1024 + kh*128 + p
"""

import numpy as np

import concourse.bass as bass
import concourse.mybir as mybir
import concourse.tile as tile
from concourse import bacc
from concourse import bass_isa
from concourse.bass_utils import run_bass_kernel_spmd

F32 = mybir.dt.float32
F16 = mybir.dt.float16
BF16 = mybir.dt.bfloat16
AF = mybir.ActivationFunctionType
OP = mybir.AluOpType
AX = mybir.AxisListType

MAGIC = 12582912.0  # 1.5 * 2**23 : (v + MAGIC) - MAGIC == rne(v) for |v| < 2**22

N_CORES = 8
RGRP = [[0, 1, 2, 3, 4, 5, 6, 7]]


def _build(T, D, H, n_cores=N_CORES, fp8=True):
    nc = bacc.Bacc("TRN2", target_bir_lowering=False, debug=False,
                   num_devices=n_cores)
    HS = H // n_cores            # gate/val row shard per core
    # host passes the weight shards PRE-TRANSPOSED (contraction-major):
    #   gate_wT/val_wT: [D, HS], out_wT: [HS, D]
    x_d = nc.dram_tensor("x", [T, D], F32, kind="ExternalInput")
    gw_d = nc.dram_tensor("gate_wT", [D, HS], F32, kind="ExternalInput")
    vw_d = nc.dram_tensor("val_wT", [D, HS], F32, kind="ExternalInput")
    ow_d = nc.dram_tensor("out_wT", [HS, D], F32, kind="ExternalInput")
    out_d = nc.dram_tensor("out", [T, D], F32, kind="ExternalOutput")

    with tile.TileContext(nc) as tc:
        _body(tc, x_d, gw_d, vw_d, ow_d, out_d, T=T, D=D, H=H,
              n_cores=n_cores, fp8=fp8)
    nc.compile()
    return nc


def _body(tc, x_d, gw_d, vw_d, ow_d, out_d, *, T, D, H, n_cores, fp8):
    nc = tc.nc
    KD = D // 128      # contraction chunks, mm1 (16)
    KH = H // 128      # contraction chunks, mm2 (64)
    NH = H // 512      # hidden 512-chunks (mm1 output tiles) (16)
    ND = D // 512      # d_out 512-chunks (mm2 output tiles) (4)
    MT = T // 128      # token chunks (8)
    HS = H // n_cores  # own gate/val rows (1024)
    RG = HS // 128     # own gate/val row-chunks (8)
    RO = D // 128      # own out_w d-row chunks (16)
    CQ = 2048          # h-quant processing chunk
    NQ = H // CQ
    MHALF = max(1, MT // 2)

    Xv = x_d.ap().rearrange("(m p) d -> m p d", p=128)
    Ov = out_d.ap().rearrange("(m p) d -> m p d", p=128)

    with (
        tc.tile_pool(name="persist", bufs=1) as pp,
        tc.tile_pool(name="psp", bufs=8, space="PSUM") as psp,
        tc.tile_pool(name="drp", bufs=1, space="DRAM") as drp,
    ):
        # DRAM scratch.  own ternary (AG inputs, pre-transposed):
        #   gv{j}_own[w, kd, p, h512]   (w: 0=gate, 1=val)
        #   oq_own[kh_local, p, d]      (own hidden cols of out_w)
        TDT = mybir.dt.float8e4 if fp8 else BF16   # gv ternary transport
        gv0_own = drp.tile([2, KD, 128, 512], TDT, tag="gv0_own")
        gv1_own = drp.tile([2, KD, 128, 512], TDT, tag="gv1_own")
        oq_own = drp.tile([HS // 128, 128, D], TDT, tag="oq_own")
        gv0_gat = drp.tile([n_cores, 2, KD, 128, 512], TDT, tag="gv0_gat",
                           addr_space="Shared")
        gv1_gat = drp.tile([n_cores, 2, KD, 128, 512], TDT, tag="gv1_gat",
                           addr_space="Shared")
        oq_gat = drp.tile([n_cores, HS // 128, 128, D], TDT, tag="oq_gat",
                          addr_space="Shared")
        ar_in = drp.tile([128, 3], F32, tag="ar_in")
        ar_out = drp.tile([128, 3], F32, tag="ar_out", addr_space="Shared")
        warm_in = drp.tile([1, 4], F32, tag="warm_in")
        warm_out = drp.tile([1, 4], F32, tag="warm_out", addr_space="Shared")
        h_d = drp.tile([MT, 128, H], F16, tag="h")

        s1, s12, gx_l, hmax = [], [], [], []
        for m in range(MT):
            for nm, lst in (("s1", s1), ("s12", s12), ("gx", gx_l),
                            ("hmax", hmax)):
                t = pp.tile([128, 1], F32, tag=f"{nm}{m}", name=f"{nm}{m}")
                lst.append(t)
        hp = [pp.tile([128, NH], F32, tag=f"hp{m}", name=f"hp{m}")
              for m in range(MT)]
        parts = pp.tile([128, 40], F32, tag="parts")
        sums = pp.tile([128, 3], F32, tag="sums")
        gsb = pp.tile([128, 3], F32, tag="gsb")
        g3 = pp.tile([128, 3], F32, tag="g3")
        thr3 = pp.tile([128, 3], F32, tag="thr3")
        nthr3 = pp.tile([128, 3], F32, tag="nthr3")

        Gv = gw_d.ap().rearrange("(r p) c -> r p c", p=128)  # [KD,128,HS]
        Vv = vw_d.ap().rearrange("(r p) c -> r p c", p=128)  # [KD,128,HS]
        Wo = ow_d.ap().rearrange("(r p) c -> r p c", p=128)  # [HS/128,128,D]

        # read views for the matmul phases
        gv0_rd = gv0_gat[:, :, :, :, :].rearrange("r w k p h -> r w p k h")
        gv1_rd = gv1_gat[:, :, :, :, :].rearrange("r w k p h -> r w p k h")
        oq_rd = oq_gat[:, :, :, :].rearrange("r k p d -> r p k d")

        with tc.tile_pool(name="kxp", bufs=1) as kxp:
            kxT = kxp.tile([128, KD, T], BF16, tag="kxT")

            # warmup collective: absorbs the CC-stack cold start while the
            # gamma DMAs stream
            with tc.tile_pool(name="wrm", bufs=1) as wrm:
                wz = wrm.tile([1, 4], F32, tag="wz")
                nc.vector.memset(wz[:, :], 0.0)
                nc.sync.dma_start(out=warm_in[:, :], in_=wz[:, :])
                nc.gpsimd.collective_compute(
                    "AllReduce", OP.add, ins=[warm_in[:, :]],
                    outs=[warm_out[:, :]], replica_groups=RGRP)

            # ------------- gamma partials + AllReduce ---------------------
            with tc.tile_pool(name="gp", bufs=3) as gp:
                def abs_chunk(src, col, W, tg):
                    wt = gp.tile([128, W], F32, tag=f"{tg}_in")
                    nc.sync.dma_start(out=wt[:, :], in_=src)
                    scr = gp.tile([128, W], BF16, tag=f"{tg}_scr", bufs=2)
                    nc.scalar.activation(out=scr[:, :], in_=wt[:, :],
                                         func=AF.Abs,
                                         accum_out=parts[:, col:col + 1])

                for r in range(KD):
                    abs_chunk(Gv[r], r, HS, "g")
                for r in range(KD):
                    abs_chunk(Vv[r], 16 + r, HS, "g")
                for r in range(HS // 128):
                    abs_chunk(Wo[r], 32 + r, D, "o")
                nc.vector.tensor_reduce(out=sums[:, 0:1], in_=parts[:, 0:16],
                                        axis=AX.X, op=OP.add)
                nc.vector.tensor_reduce(out=sums[:, 1:2],
                                        in_=parts[:, 16:32],
                                        axis=AX.X, op=OP.add)
                nc.vector.tensor_reduce(out=sums[:, 2:3],
                                        in_=parts[:, 32:40],
                                        axis=AX.X, op=OP.add)
                nc.sync.dma_start(out=ar_in[:, :], in_=sums[:, :])

            # ------------- x quantization + transpose ---------------------
            # kxT[p=d, k, t] = k_x[t, k*128+p]
            with tc.tile_pool(name="xp", bufs=3) as xp:
                for m in range(MT):
                    xt = xp.tile([128, D], F32, tag="x_in")
                    nc.sync.dma_start(out=xt[:, :], in_=Xv[m])
                    gx = gx_l[m]
                    nc.vector.tensor_reduce(out=gx[:, :], in_=xt[:, :],
                                            axis=AX.X, op=OP.max,
                                            apply_absolute_value=True)
                    nc.vector.tensor_scalar_max(out=gx[:, :], in0=gx[:, :],
                                                scalar1=1e-5)
                    rcp = xp.tile([128, 1], F32, tag="rcpx")
                    nc.vector.reciprocal(out=rcp[:, :], in_=gx[:, :])
                    sx = xp.tile([128, 1], F32, tag="sx")
                    nc.vector.tensor_scalar_mul(out=sx[:, :], in0=rcp[:, :],
                                                scalar1=127.0)
                    xs = xp.tile([128, D], F32, tag="x_sc")
                    nc.scalar.activation(out=xs[:, :], in_=xt[:, :],
                                         func=AF.Copy, scale=sx[:, :])
                    kx = xp.tile([128, D], BF16, tag="kx")
                    nc.vector.tensor_scalar(out=kx[:, :], in0=xs[:, :],
                                            scalar1=MAGIC, scalar2=MAGIC,
                                            op0=OP.add, op1=OP.subtract)
                    nc.sync.dma_start(out=kxT[:, :, m * 128:(m + 1) * 128],
                                      in_=kx[:, :], transpose=True)

            # ------------- scales + ternarize own shards + AllGather ------
            with tc.tile_pool(name="wp", bufs=3) as wp:
                # gamma AllReduce (collective-stack warm by now)
                nc.gpsimd.collective_compute(
                    "AllReduce", OP.add, ins=[ar_in[:, :]],
                    outs=[ar_out[:, :]], replica_groups=RGRP)
                nc.sync.dma_start(out=gsb[:, :], in_=ar_out[:, :])
                nc.gpsimd.partition_all_reduce(gsb[:, :], gsb[:, :], 128,
                                               bass_isa.ReduceOp.add)
                # gamma = clip(mean, 1e-5); thr = 0.5*gamma
                nc.vector.tensor_scalar(out=g3[:, :], in0=gsb[:, :],
                                        scalar1=1.0 / (H * D),
                                        scalar2=1e-5, op0=OP.mult,
                                        op1=OP.max)
                nc.vector.tensor_scalar_mul(out=thr3[:, :], in0=g3[:, :],
                                            scalar1=0.5)
                nc.vector.tensor_scalar_mul(out=nthr3[:, :], in0=thr3[:, :],
                                            scalar1=-1.0)
                # per-token eviction scales; thr/127 == gamma/254 folds the
                # ternary 2x.  s12[m] here holds the VAL scale s2.
                for m in range(MT):
                    nc.vector.tensor_scalar(out=s1[m][:, :],
                                            in0=gx_l[m][:, :],
                                            scalar1=thr3[:, 0:1],
                                            scalar2=1.0 / 127.0,
                                            op0=OP.mult, op1=OP.mult)
                    nc.vector.tensor_scalar(out=s12[m][:, :],
                                            in0=gx_l[m][:, :],
                                            scalar1=thr3[:, 1:2],
                                            scalar2=1.0 / 127.0,
                                            op0=OP.mult, op1=OP.mult)

                # ternarize one [128, W] chunk -> {-2,0,2} tile (dt=odt);
                # loads ride the SWDGE/gpsimd queue to overlap the sync queue
                def tern_chunk(src, W, tg, thr, nthr, dve, odt=BF16,
                               out_ap=None, in_bufs=3):
                    wt = wp.tile([128, W], F32, tag=f"{tg}_in", bufs=in_bufs)
                    nc.sync.dma_start(out=wt[:, :], in_=src)
                    if out_ap is None:
                        tq = wp.tile([128, W], odt, tag=f"{tg}_tq")
                    else:
                        tq = None
                    if dve:
                        mp = wp.tile([128, W], BF16, tag=f"{tg}_mp", bufs=3)
                        nc.vector.tensor_scalar(out=mp[:, :], in0=wt[:, :],
                                                scalar1=thr, scalar2=2.0,
                                                op0=OP.is_gt, op1=OP.mult)
                        mn = wp.tile([128, W], BF16, tag=f"{tg}_mn", bufs=3)
                        nc.vector.tensor_scalar(out=mn[:, :], in0=wt[:, :],
                                                scalar1=nthr, scalar2=2.0,
                                                op0=OP.is_lt, op1=OP.mult)
                        dst = tq[:, :] if out_ap is None else out_ap
                        nc.vector.tensor_sub(out=dst, in0=mp[:, :],
                                             in1=mn[:, :])
                    else:
                        sp = wp.tile([128, W], BF16, tag=f"{tg}_sp", bufs=3)
                        nc.scalar.activation(out=sp[:, :], in_=wt[:, :],
                                             func=AF.Sign, bias=nthr)
                        sn = wp.tile([128, W], BF16, tag=f"{tg}_sn", bufs=3)
                        nc.scalar.activation(out=sn[:, :], in_=wt[:, :],
                                             func=AF.Sign, bias=thr)
                        dst = tq[:, :] if out_ap is None else out_ap
                        nc.vector.tensor_add(out=dst, in0=sp[:, :],
                                             in1=sn[:, :])
                    return tq

                t_g, nt_g = thr3[:, 0:1], nthr3[:, 0:1]
                t_v, nt_v = thr3[:, 1:2], nthr3[:, 1:2]
                t_o, nt_o = thr3[:, 2:3], nthr3[:, 2:3]

                # gate/val: natural writes, even/odd h-halves -> AG#0/AG#1
                for r in range(KD):
                    tqg = tern_chunk(Gv[r], HS, "g", t_g, nt_g, False,
                                     odt=TDT, in_bufs=10)
                    nc.sync.dma_start(out=gv0_own[0, r, :, :],
                                      in_=tqg[:, 0:HS // 2])
                    nc.sync.dma_start(out=gv1_own[0, r, :, :],
                                      in_=tqg[:, HS // 2:HS])
                    tqv = tern_chunk(Vv[r], HS, "g", t_v, nt_v, True,
                                     odt=TDT, in_bufs=10)
                    nc.sync.dma_start(out=gv0_own[1, r, :, :],
                                      in_=tqv[:, 0:HS // 2])
                    nc.sync.dma_start(out=gv1_own[1, r, :, :],
                                      in_=tqv[:, HS // 2:HS])
                nc.gpsimd.collective_compute(
                    "AllGather", OP.bypass, ins=[gv0_own[:, :, :, :]],
                    outs=[gv0_gat[:, :, :, :, :]], replica_groups=RGRP)
                nc.gpsimd.collective_compute(
                    "AllGather", OP.bypass, ins=[gv1_own[:, :, :, :]],
                    outs=[gv1_gat[:, :, :, :, :]], replica_groups=RGRP)

                # out_w own columns (pre-transposed): natural writes
                for r in range(HS // 128):
                    tqo = tern_chunk(Wo[r], D, "o", t_o, nt_o,
                                     dve=(r % 2 == 1), odt=TDT, in_bufs=4)
                    nc.sync.dma_start(out=oq_own[r, :, :], in_=tqo[:, :])
                nc.gpsimd.collective_compute(
                    "AllGather", OP.bypass, ins=[oq_own[:, :, :]],
                    outs=[oq_gat[:, :, :, :]], replica_groups=RGRP)

            # ---------------- mm1: gate/val matmuls + h ----------------
            gat_rd = [gv0_rd, gv1_rd]
            order = [2 * r for r in range(NH // 2)] + \
                    [2 * r + 1 for r in range(NH // 2)]
            with tc.tile_pool(name="m1p", bufs=2) as m1p:
                for n in order:
                    j, r = n % 2, n // 2
                    # natural bulk loads of pre-transposed weights; the
                    # fp8 ternary feeds the PE directly (mixed-dtype matmul)
                    wg_n = m1p.tile([128, KD, 512], TDT, tag="wg_n",
                                    bufs=3)
                    nc.sync.dma_start(out=wg_n[:, :, :],
                                      in_=gat_rd[j][r, 0])
                    wv_n = m1p.tile([128, KD, 512], TDT, tag="wv_n",
                                    bufs=3)
                    nc.sync.dma_start(out=wv_n[:, :, :],
                                      in_=gat_rd[j][r, 1])
                    for hf in range(MT // MHALF):
                        ms = range(hf * MHALF, (hf + 1) * MHALF)
                        pg = {m: psp.tile([128, 512], F32, tag="ps",
                                          name=f"pg{n}_{m}") for m in ms}
                        pv = {m: psp.tile([128, 512], F32, tag="ps",
                                          name=f"pv{n}_{m}") for m in ms}
                        for k in range(KD):
                            for m in ms:
                                lhsT = kxT[:, k, m * 128:(m + 1) * 128]
                                nc.tensor.matmul(pg[m][:, :], lhsT=lhsT,
                                                 rhs=wg_n[:, k, :],
                                                 start=(k == 0),
                                                 stop=(k == KD - 1))
                                nc.tensor.matmul(pv[m][:, :], lhsT=lhsT,
                                                 rhs=wv_n[:, k, :],
                                                 start=(k == 0),
                                                 stop=(k == KD - 1))
                        for m in ms:
                            # h = silu(pg*s1) * (pv*s2): one ACT + two DVE
                            A = m1p.tile([128, 512], F32, tag="Asb",
                                         bufs=MHALF + 2, name=f"A{n}_{m}")
                            nc.scalar.activation(out=A[:, :], in_=pg[m][:, :],
                                                 func=AF.Silu,
                                                 scale=s1[m][:, :])
                            tmp = m1p.tile([128, 512], F32, tag="tmp", bufs=4,
                                           name=f"tmp{n}_{m}")
                            nc.vector.tensor_scalar_mul(out=tmp[:, :],
                                                        in0=pv[m][:, :],
                                                        scalar1=s12[m][:, :])
                            hs = m1p.tile([128, 512], F16, tag="hsl", bufs=4,
                                          name=f"hs{n}_{m}")
                            nc.vector.tensor_mul(out=hs[:, :], in0=A[:, :],
                                                 in1=tmp[:, :])
                            nc.vector.tensor_reduce(
                                out=hp[m][:, n:n + 1], in_=hs[:, :],
                                axis=AX.X, op=OP.max,
                                apply_absolute_value=True)
                            nc.sync.dma_start(
                                out=h_d[m, :, n * 512:(n + 1) * 512],
                                in_=hs[:, :])

        # ---------------- h quantization + mm2 (interleaved) ----------
        with (
            tc.tile_pool(name="khp", bufs=1) as khp,
            tc.tile_pool(name="hqp", bufs=2) as hqp,
            tc.tile_pool(name="m2p", bufs=2) as m2p,
        ):
            khT, s_out = [], []
            NQT = 8                    # k-groups per c (one rank each)
            KQ = KH // NQT             # 8 k-chunks per group
            wo_pref = {}
            for q in range(2):         # prefetch c=0 weight tiles early
                wt0 = m2p.tile([128, KQ, 512], TDT, tag="wo_q", bufs=4,
                               name=f"wo_pref{q}")
                nc.sync.dma_start(out=wt0[:, :, :],
                                  in_=oq_rd[q][:, :, 0:512])
                wo_pref[(0, q)] = wt0
            for m in range(MT):
                nc.vector.tensor_reduce(out=hmax[m][:, :],
                                        in_=hp[m][:, :], axis=AX.X,
                                        op=OP.max)
                gh = hqp.tile([128, 1], F32, tag="gh")
                nc.vector.tensor_scalar_max(out=gh[:, :],
                                            in0=hmax[m][:, :],
                                            scalar1=1e-5)
                rch = hqp.tile([128, 1], F32, tag="rch")
                nc.vector.reciprocal(out=rch[:, :], in_=gh[:, :])
                sh = hqp.tile([128, 1], F32, tag="sh")
                nc.vector.tensor_scalar_mul(out=sh[:, :], in0=rch[:, :],
                                            scalar1=127.0)
                so = pp.tile([128, 1], F32, tag=f"so{m}", name=f"so{m}")
                nc.vector.tensor_scalar(out=so[:, :], in0=gh[:, :],
                                        scalar1=thr3[:, 2:3],
                                        scalar2=1.0 / 127.0,
                                        op0=OP.mult, op1=OP.mult)
                s_out.append(so)
                kT = khp.tile([128, KH, 128], BF16, tag=f"khT{m}",
                              name=f"khT{m}")
                khT.append(kT)
                # k_h = rne(h*sh); the (h*sh + MAGIC) pass alternates
                # between ScalarE and VectorE, and each kh half is
                # transposed as soon as it is complete
                kh_full = hqp.tile([128, H], BF16, tag="kh_full", bufs=1)
                for q in range(2):
                    HH = H // 2
                    hc = hqp.tile([128, HH], F16, tag="h_rd")
                    nc.scalar.dma_start(out=hc[:, :],
                                        in_=h_d[m, :, q * HH:(q + 1) * HH])
                    for u in range(HH // CQ):
                        sli = slice(u * CQ, (u + 1) * CQ)
                        slo = slice(q * HH + u * CQ, q * HH + (u + 1) * CQ)
                        tmp = hqp.tile([128, CQ], F32, tag="h_tmp")
                        if (q * (HH // CQ) + u) % 2 == 0:
                            nc.vector.tensor_scalar(out=tmp[:, :],
                                                    in0=hc[:, sli],
                                                    scalar1=sh[:, :],
                                                    scalar2=MAGIC,
                                                    op0=OP.mult, op1=OP.add)
                        else:
                            nc.scalar.activation(out=tmp[:, :],
                                                 in_=hc[:, sli],
                                                 func=AF.Copy,
                                                 scale=sh[:, :],
                                                 bias=MAGIC)
                        nc.vector.tensor_scalar_sub(out=kh_full[:, slo],
                                                    in0=tmp[:, :],
                                                    scalar1=MAGIC)
                    nc.sync.dma_start(
                        out=kT[:, q * (KH // 2):(q + 1) * (KH // 2), :],
                        in_=kh_full[:, q * HH:(q + 1) * HH],
                        transpose=True)

            # mm2: c outer, k-quarters, m-chains -- tensor engine starts as
            # soon as khT[0] is ready; wo loads are natural bulk reads
            for c in range(ND):
                csl = slice(c * 512, (c + 1) * 512)
                po = [psp.tile([128, 512], F32, tag="ps",
                               name=f"po{c}_{m}") for m in range(MT)]
                for q in range(NQT):
                    if (c, q) in wo_pref:
                        wo_q = wo_pref[(c, q)]
                    else:
                        wo_q = m2p.tile([128, KQ, 512], TDT, tag="wo_q",
                                        bufs=4)
                        nc.sync.dma_start(out=wo_q[:, :, :],
                                          in_=oq_rd[q][:, :, csl])
                    for m in range(MT):
                        for kk in range(KQ):
                            k = q * KQ + kk
                            nc.tensor.matmul(po[m][:, :],
                                             lhsT=khT[m][:, k, :],
                                             rhs=wo_q[:, kk, :],
                                             start=(k == 0),
                                             stop=(k == KH - 1))
                for m in range(MT):
                    ot = m2p.tile([128, 512], F32, tag="ot", bufs=4,
                                  name=f"ot{c}_{m}")
                    nc.scalar.activation(out=ot[:, :], in_=po[m][:, :],
                                         func=AF.Copy,
                                         scale=s_out[m][:, :])
                    nc.sync.dma_start(out=Ov[m][:, csl], in_=ot[:, :])


_NC_CACHE = {}


def _get_nc(T, D, H):
    key = (T, D, H)
    if key not in _NC_CACHE:
        try:
            _NC_CACHE[key] = _build(T, D, H, fp8=True)
        except Exception:
            _NC_CACHE[key] = _build(T, D, H, fp8=False)
    return _NC_CACHE[key]


def kernel(x, gate_w, gate_b, val_w, val_b, out_w, out_b, _trace=False):
    x = np.ascontiguousarray(np.asarray(x), dtype=np.float32)
    gate_w = np.ascontiguousarray(np.asarray(gate_w), dtype=np.float32)
    val_w = np.ascontiguousarray(np.asarray(val_w), dtype=np.float32)
    out_w = np.ascontiguousarray(np.asarray(out_w), dtype=np.float32)
    gate_b = np.asarray(gate_b)
    val_b = np.asarray(val_b)
    out_b = np.asarray(out_b)
    assert not np.any(gate_b) and not np.any(val_b), (
        "device kernel folds silu(y+b) with b=0; nonzero gate/val bias "
        "not supported")

    orig_shape = x.shape
    xf = x.reshape(-1, x.shape[-1])
    n_tok, d = xf.shape
    h = gate_w.shape[0]
    t_core = n_tok // N_CORES
    hs = h // N_CORES

    nc = _get_nc(t_core, d, h)
    in_maps = [
        {
            "x": xf[i * t_core:(i + 1) * t_core],
            "gate_wT": np.ascontiguousarray(gate_w[i * hs:(i + 1) * hs].T),
            "val_wT": np.ascontiguousarray(val_w[i * hs:(i + 1) * hs].T),
            "out_wT": np.ascontiguousarray(out_w[:, i * hs:(i + 1) * hs].T),
        }
        for i in range(N_CORES)
    ]
    res = run_bass_kernel_spmd(nc, in_maps, core_ids=list(range(N_CORES)),
                               trace=_trace)
    out = np.concatenate([res.results[i]["out"] for i in range(N_CORES)],
                         axis=0)
    out = out + out_b[None, :].astype(np.float32)
    kernel._last_results = res
    return out.reshape(orig_shape)
